# revision 7
# baseline (speedup 1.0000x reference)
"""Trainium2 Bass kernel for nn_DeformableConvLSTMCell_33895881900284.

Full (unsharded) inputs in, full outputs out. Internally: data-parallel over
batch across 8 NeuronCores (8 batches per core), conv weights / gate params
replicated.

Math per the reference:
  outI  = conv3x3_same(inputs, wconvInput)
  g     = tanh(outI + conv3x3_same(hidden_prev, wconvHidden) + gateBias)
  gapI  = mean_hw(outI);  gapH = mean_hw(hidden_prev)          # [B, D]
  i/f/o = sigmoid(wx*gapI + wh*gapH + bias)                    # [B, D]
  tiled gate: value used at (b, h, w, c) is gate[(28*b + h) % 64, c]
  state  = f*state_prev + i*g;  hidden = o*tanh(state)

The (28*b+h)%64 scrambling makes gates cross-batch: each core computes its
local GAP columns, all cores AllGather them, and a per-core index-array input
drives an indirect-DMA gather of exactly the gate rows this core's outputs
need (so the SPMD program stays identical across cores).

gapI is computed without touching the conv output, via linearity:
  784*gapI = S@A_S - Rf@A_rf - Rl@A_rl - Cf@A_cf - Cl@A_cl + corner terms
where S/Rf/Rl/Cf/Cl are full/row/col sums of the input image and A_* are
sums of conv-weight taps. This keeps the conv pipeline free of mid-stream
PSUM reads.

Matmuls run in float32r (full-rate fp32 with TF32-like operand rounding);
operands are rounded by the producing DVE copy. Conv is expressed as 9
shifted matmuls accumulating in PSUM over a zero-padded, transposed
([channel, pixel]) image layout (30x30 incl. 1-pixel halo).
"""
import numpy as np

import bass_rust
import concourse.bass as bass
import concourse.mybir as mybir
import concourse.tile as tile
from concourse.bass_utils import run_bass_kernel_spmd

F32 = mybir.dt.float32
F32R = mybir.dt.float32r
I32 = mybir.dt.int32
AF = mybir.ActivationFunctionType
ALU = mybir.AluOpType

N_CORES = 8
B, H, W, CIN, D = 64, 28, 28, 256, 256
BL = B // N_CORES          # local batches per core
PIX = H * W                # 784
PG = 112                   # pixels per transpose group (4 rows)
NPG = PIX // PG            # 7
PAD = 30                   # padded row/col length
XTLEN = PAD * PAD          # 900
NW = 2                     # windows per batch
WROWS = H // NW            # 14
WN = WROWS * W             # 392
NCC = CIN // 128           # 2 channel chunks
NDC = D // 128             # 2 output-channel chunks

# tap order t = 3*kh + kw ; dh = kh-1, dw = kw-1
TAPS = [(kh, kw) for kh in range(3) for kw in range(3)]

# ---------------------------------------------------------------------------
# walrus fixup: split semaphore waits that exceed the per-instruction budget
# (Drain takes only 1; regular instructions also capped conservatively at 1).
MAX_WAITS = 1


def _split_excess_sem_waits(nc):
    counter = [0]
    for fn in nc.m.functions:
        for bb in fn.blocks:
            insts = bb.instructions
            i = 0
            while i < len(insts):
                inst = insts[i]
                si = inst.sync_info
                if si is not None and si.on_wait and len(si.on_wait) > MAX_WAITS:
                    waits = list(si.on_wait)
                    excess = waits[:-MAX_WAITS]
                    keep = waits[-MAX_WAITS:]
                    new_insts = []
                    for j in range(0, len(excess), MAX_WAITS):
                        chunk = excess[j:j + MAX_WAITS]
                        noop = mybir.InstNoOp(
                            name=f"I-waitsplit-{counter[0]}", ins=[], outs=[])
                        counter[0] += 1
                        noop.engine = inst.engine
                        noop.sync_info = bass_rust.SyncInfo(
                            on_wait=chunk, on_update=[])
                        nc.register_instruction(noop)
                        new_insts.append(noop)
                    inst.sync_info = bass_rust.SyncInfo(
                        on_wait=keep, on_update=list(si.on_update))
                    insts[i:i] = new_insts
                    i += len(new_insts)
                i += 1
    return nc


# ---------------------------------------------------------------------------
def _r3(ap, w):
    """view flat free dim as (rows, w)"""
    return ap.rearrange("c (r w) -> c r w", w=w)


def build_nc():
    nc = bass.Bass("TRN2", target_bir_lowering=False, debug=False,
                   num_devices=N_CORES)

    # ---- DRAM I/O (per-core shard shapes) ----
    d_in = nc.dram_tensor("inputs", [BL, PIX, CIN], F32, kind="ExternalInput").ap()
    d_sp = nc.dram_tensor("state_prev", [BL, PIX, D], F32, kind="ExternalInput").ap()
    d_hp = nc.dram_tensor("hidden_prev", [BL, PIX, D], F32, kind="ExternalInput").ap()
    d_w1 = nc.dram_tensor("wconvInput", [3, 3, CIN, D], F32, kind="ExternalInput").ap()
    d_w2 = nc.dram_tensor("wconvHidden", [3, 3, D, D], F32, kind="ExternalInput").ap()
    d_gb = nc.dram_tensor("gateBias", [PIX, D], F32, kind="ExternalInput").ap()
    d_vec = {}
    for nm in ("wxi", "whi", "inputBias", "wxf", "whf", "forgetBias",
               "wxo", "who", "outputBias"):
        d_vec[nm] = nc.dram_tensor(nm, [D, 1], F32, kind="ExternalInput").ap()
    d_id = nc.dram_tensor("identity", [128, 128], F32, kind="ExternalInput").ap()
    d_idx = nc.dram_tensor("gate_idx", [4, 224, 1], I32, kind="ExternalInput").ap()
    d_hid = nc.dram_tensor("hidden", [BL, PIX, D], F32, kind="ExternalOutput").ap()
    d_st = nc.dram_tensor("state", [BL, PIX, D], F32, kind="ExternalOutput").ap()

    cc_in = nc.dram_tensor("cc_in", [32, 128], F32, kind="Internal").ap()
    cc_out = nc.dram_tensor("cc_out", [N_CORES * 32, 128], F32, kind="Internal",
                            addr_space="Shared").ap()

    ctx_mgr = nc.allow_low_precision("float32r operand rounding for PE")
    ctx_mgr.__enter__()
    with tile.TileContext(nc) as tc:
        _build_body(nc, tc, d_in, d_sp, d_hp, d_w1, d_w2, d_gb, d_vec, d_id,
                    d_idx, d_hid, d_st, cc_in, cc_out)
    ctx_mgr.__exit__(None, None, None)
    return nc


def _build_body(nc, tc, d_in, d_sp, d_hp, d_w1, d_w2, d_gb, d_vec, d_id,
                d_idx, d_hid, d_st, cc_in, cc_out):
    from contextlib import ExitStack
    ctx = ExitStack()
    pool = lambda **kw: ctx.enter_context(tc.tile_pool(**kw))

    const = pool(name="const", bufs=1)
    wts = pool(name="wts", bufs=1)
    stage = pool(name="stage", bufs=4)
    xt_in = pool(name="xt_in", bufs=3)     # per-cc tag; 3 batches in flight
    xt_hid = pool(name="xt_hid", bufs=3)
    xt_sp = pool(name="xt_sp", bufs=2)
    ew = pool(name="ew", bufs=3)           # elementwise temporaries
    outb = pool(name="outb", bufs=4)       # stateT/hiddenT full-batch buffers
    nato = pool(name="nato", bufs=4)       # natural-layout output staging
    gsm = pool(name="gsm", bufs=1)         # persistent small gate/gap tiles
    gtmp = pool(name="gtmp", bufs=2)       # gate-math temporaries
    ps_conv = pool(name="ps_conv", bufs=3, space="PSUM")
    ps_tr = pool(name="ps_tr", bufs=2, space="PSUM")
    ps_gap = pool(name="ps_gap", bufs=1, space="PSUM")
    gdram = pool(name="gdram", bufs=2 * BL, space="DRAM")

    # ================= phase 0: constants =================
    ident = const.tile([128, 128], F32, tag="ident")
    nc.sync.dma_start(ident[:], d_id[:])

    # gate vectors -> [128,1] per chunk; wx/wh scaled by 1/PIX
    vecs = {}
    for nm in d_vec:
        t = const.tile([128, NDC], F32, tag=f"vec_{nm}")
        for c in range(NDC):
            nc.sync.dma_start(t[:, c:c + 1], d_vec[nm][c * 128:(c + 1) * 128, :])
        if nm.startswith("wx") or nm.startswith("wh"):
            nc.vector.tensor_scalar_mul(t[:], t[:], 1.0 / PIX)
        vecs[nm] = t

    # conv weights -> f32r stationary tiles; layout [128, (conv*9+t)*NCC+cc -> 256]
    wconv = wts.tile([128, 2 * 9 * NCC * 256], F32R, tag="wconv")

    def wblk(conv, t, cc):
        off = ((conv * 9 + t) * NCC + cc) * 256
        return wconv[:, off:off + 256]

    for conv, dw in ((0, d_w1), (1, d_w2)):
        for t, (kh, kw) in enumerate(TAPS):
            for cc in range(NCC):
                ws = stage.tile([128, 256], F32, tag="wstage")
                nc.sync.dma_start(ws[:], dw[kh, kw, cc * 128:(cc + 1) * 128, :])
                nc.vector.tensor_copy(wblk(conv, t, cc), ws[:])

    # combined A-tiles for gapI (from conv1 weights), 9 groups per cc
    # group order: S, Rf, Rl, Cf, Cl, K00, K0L, KL0, KLL
    a_r = wts.tile([128, NCC * 9 * 256], F32R, tag="a_r")

    def ablk(cc, g):
        off = (cc * 9 + g) * 256
        return a_r[:, off:off + 256]

    for cc in range(NCC):
        # A_S = sum of all 9 taps
        nc.vector.tensor_copy(ablk(cc, 0), wblk(0, 0, cc))
        for t in range(1, 9):
            nc.vector.tensor_tensor(out=ablk(cc, 0), in0=ablk(cc, 0),
                                    in1=wblk(0, t, cc), op=ALU.add)
        # A_Rf_neg = -(taps kh=2) ; A_Rl_neg = -(taps kh=0)
        # A_Cf_neg = -(taps kw=2) ; A_Cl_neg = -(taps kw=0)
        for g, taps in ((1, [6, 7, 8]), (2, [0, 1, 2]),
                        (3, [2, 5, 8]), (4, [0, 3, 6])):
            nc.vector.tensor_copy(ablk(cc, g), wblk(0, taps[0], cc))
            for t in taps[1:]:
                nc.vector.tensor_tensor(out=ablk(cc, g), in0=ablk(cc, g),
                                        in1=wblk(0, t, cc), op=ALU.add)
            nc.vector.tensor_scalar_mul(ablk(cc, g), ablk(cc, g), -1.0)
        # corners: K00 -> tap (2,2)=8 ; K0L -> (2,0)=6 ; KL0 -> (0,2)=2 ; KLL -> (0,0)=0
        for g, t in ((5, 8), (6, 6), (7, 2), (8, 0)):
            nc.vector.tensor_copy(ablk(cc, g), wblk(0, t, cc))

    # gateBias transposed: [128 d, PIX] per dc
    gbias = [const.tile([128, PIX], F32, tag=f"gbias{dc}", name=f"gbias{dc}")
             for dc in range(NDC)]
    for g7 in range(NPG):
        nat = stage.tile([128, 256], F32, tag="natload")
        nc.sync.dma_start(nat[0:PG, :], d_gb[g7 * PG:(g7 + 1) * PG, :])
        for dc in range(NDC):
            pt = ps_tr.tile([128, PG], F32, tag="ptr")
            nc.tensor.transpose(pt[:], nat[0:PG, dc * 128:(dc + 1) * 128],
                                ident[0:PG, 0:PG])
            nc.vector.tensor_copy(gbias[dc][:, g7 * PG:(g7 + 1) * PG], pt[:])

    # gather index tiles
    idx_sb = []
    for g4 in range(4):
        halves = []
        for hf in range(2):
            t = const.tile([PG, 1], I32, tag=f"idx{g4}_{hf}")
            nc.sync.dma_start(t[:], d_idx[g4, hf * PG:(hf + 1) * PG, :])
            halves.append(t)
        idx_sb.append(halves)

    # gap staging
    raw = [gsm.tile([128, 9 * BL], F32R, tag=f"raw{cc}", name=f"raw{cc}")
           for cc in range(NCC)]
    gapH = [gsm.tile([128, BL], F32, tag=f"gapH{cc}", name=f"gapHs{cc}")
            for cc in range(NCC)]

    # ================= phase 1: per-batch transposes + raws + convs ========
    def load_transposed(j, dsrc, xpool, tagbase, dtype, padded):
        """DMA [PIX, 256] natural -> transpose -> [128, XTLEN|PIX] per cc."""
        tiles = []
        for cc in range(NCC):
            xlen = XTLEN if padded else PIX
            xt = xpool.tile([128, xlen], dtype, tag=f"{tagbase}{cc}")
            if padded:
                # zero the halo: rows 0 & 29 + cols 0 & 29 (memset wants a
                # plain dtype, so view the f32r tile as f32)
                x3 = _r3(xt[:].bitcast(F32), PAD)
                nc.gpsimd.memset(x3[:, 0:1, :], 0.0)
                nc.gpsimd.memset(x3[:, PAD - 1:PAD, :], 0.0)
                nc.gpsimd.memset(x3[:, 1:PAD - 1, 0:1], 0.0)
                nc.gpsimd.memset(x3[:, 1:PAD - 1, PAD - 1:PAD], 0.0)
            tiles.append(xt)
        for g7 in range(NPG):
            nat = stage.tile([128, 256], F32, tag="natload")
            nc.sync.dma_start(nat[0:PG, :], dsrc[j, g7 * PG:(g7 + 1) * PG, :])
            for cc in range(NCC):
                pt = ps_tr.tile([128, PG], F32, tag="ptr")
                nc.tensor.transpose(pt[:], nat[0:PG, cc * 128:(cc + 1) * 128],
                                    ident[0:PG, 0:PG])
                if padded:
                    dst = _r3(tiles[cc][:], PAD)[:, 1 + 4 * g7:1 + 4 * g7 + 4, 1:29]
                else:
                    dst = _r3(tiles[cc][:, g7 * PG:(g7 + 1) * PG], W)
                nc.vector.tensor_copy(dst, pt[:].rearrange("c (r w) -> c r w", w=W))
        return tiles

    g_parked = {}
    for j in range(BL):
        xin = load_transposed(j, d_in, xt_in, "xin", F32R, True)
        xhid = load_transposed(j, d_hp, xt_hid, "xhid", F32R, True)

        # raw gap sums from inputs
        for cc in range(NCC):
            xt = xin[cc]
            x3 = _r3(xt[:], PAD)

            def col(g):
                return raw[cc][:, g * BL + j:g * BL + j + 1]

            nc.vector.tensor_reduce(col(0), xt[:], axis=mybir.AxisListType.X,
                                    op=ALU.add)
            nc.vector.tensor_reduce(col(1), xt[:, 31:59],
                                    axis=mybir.AxisListType.X, op=ALU.add)
            nc.vector.tensor_reduce(col(2), xt[:, 841:869],
                                    axis=mybir.AxisListType.X, op=ALU.add)
            nc.vector.tensor_reduce(col(3), x3[:, 1:29, 1:2],
                                    axis=mybir.AxisListType.XY, op=ALU.add)
            nc.vector.tensor_reduce(col(4), x3[:, 1:29, 28:29],
                                    axis=mybir.AxisListType.XY, op=ALU.add)
            nc.vector.tensor_copy(col(5), xt[:, 31:32])     # (1,1)
            nc.vector.tensor_copy(col(6), xt[:, 58:59])     # (1,28)
            nc.vector.tensor_copy(col(7), xt[:, 841:842])   # (28,1)
            nc.vector.tensor_copy(col(8), xt[:, 868:869])   # (28,28)
            # gapH: full-buffer sum of hidden (halo zeros don't matter)
            nc.vector.tensor_reduce(gapH[cc][:, j:j + 1], xhid[cc][:],
                                    axis=mybir.AxisListType.X, op=ALU.add)

        # conv windows
        for wi in range(NW):
            h0 = 1 + wi * WROWS
            for dc in range(NDC):
                p = ps_conv.tile([128, WN], F32, tag="pconv")
                p3 = _r3(p[:], W)
                first = True
                for conv, xbuf in ((0, xin), (1, xhid)):
                    for t, (kh, kw) in enumerate(TAPS):
                        dh, dwid = kh - 1, kw - 1
                        for cc in range(NCC):
                            rhs = _r3(xbuf[cc][:], PAD)[
                                :, h0 + dh:h0 + dh + WROWS,
                                1 + dwid:1 + dwid + W]
                            last = (conv == 1 and t == 8 and cc == NCC - 1)
                            nc.tensor.matmul(
                                p3, wblk(conv, t, cc)[:, dc * 128:(dc + 1) * 128],
                                rhs, start=first, stop=last)
                            first = False
                # g = tanh(conv + gateBias)
                t0 = ew.tile([128, WN], F32, tag="t0")
                base = (h0 - 1) * W
                nc.vector.tensor_tensor(out=t0[:], in0=p[:],
                                        in1=gbias[dc][:, base:base + WN],
                                        op=ALU.add)
                gt = ew.tile([128, WN], F32, tag="gt")
                nc.scalar.activation(gt[:], t0[:], AF.Tanh)
                key = (j, dc)
                if key not in g_parked:
                    g_parked[key] = gdram.tile([128, PIX], F32, tag="gpark",
                                               name=f"gpark_{j}_{dc}")
                nc.sync.dma_start(g_parked[key][:, base:base + WN], gt[:])

    # ================= phase 2: gap -> collective -> gates =================
    gap_ps = ps_gap.tile([8, 256], F32, tag="gapI")
    for cc in range(NCC):
        for g in range(9):
            nc.tensor.matmul(gap_ps[:], raw[cc][:, g * BL:(g + 1) * BL],
                             ablk(cc, g),
                             start=(cc == 0 and g == 0),
                             stop=(cc == NCC - 1 and g == 8))
    gapI_sb = gsm.tile([8, 256], F32, tag="gapI_sb")
    nc.vector.tensor_copy(gapI_sb[:], gap_ps[:])
    nc.sync.dma_start(cc_in[0:8, :], gapI_sb[:, 0:128])
    nc.sync.dma_start(cc_in[8:16, :], gapI_sb[:, 128:256])
    for cc in range(NCC):
        pt = ps_gap.tile([8, 128], F32, tag="gapHt")
        nc.tensor.transpose(pt[:], gapH[cc][:], ident[:])
        hs = gsm.tile([8, 128], F32, tag=f"gapH_sb{cc}")
        nc.vector.tensor_copy(hs[:], pt[:])
        nc.sync.dma_start(cc_in[16 + 8 * cc:24 + 8 * cc, :], hs[:])

    nc.gpsimd.collective_compute(
        "AllGather", ALU.bypass, replica_groups=[list(range(N_CORES))],
        ins=[cc_in[:]], outs=[cc_out[:]])

    # gather the per-(b,h) gap rows, transpose to [128 ch, 224]
    sel = [gsm.tile([128, 224], F32, tag=f"sel{g4}", name=f"sel{g4}")
           for g4 in range(4)]
    for g4 in range(4):
        for hf in range(2):
            gtile = stage.tile([PG, 128], F32, tag="gath")
            nc.gpsimd.indirect_dma_start(
                out=gtile[:], out_offset=None, in_=cc_out[:],
                in_offset=bass.IndirectOffsetOnAxis(ap=idx_sb[g4][hf][:, :1],
                                                    axis=0))
            pt = ps_tr.tile([128, PG], F32, tag="ptr")
            nc.tensor.transpose(pt[:], gtile[:], ident[0:PG, 0:PG])
            nc.vector.tensor_copy(sel[g4][:, hf * PG:(hf + 1) * PG], pt[:])

    # gates: sel[0/1] = gapI sums (dc0/dc1); sel[2/3] = gapH sums
    gates = {}
    for gate, wx, wh, bi in (("i", "wxi", "whi", "inputBias"),
                             ("f", "wxf", "whf", "forgetBias"),
                             ("o", "wxo", "who", "outputBias")):
        per_dc = []
        for dc in range(NDC):
            t1 = gtmp.tile([128, 224], F32, tag="gm1")
            nc.vector.tensor_scalar_mul(t1[:], sel[dc][:],
                                        vecs[wx][:, dc:dc + 1])
            t2 = gtmp.tile([128, 224], F32, tag="gm2")
            nc.vector.tensor_scalar_mul(t2[:], sel[2 + dc][:],
                                        vecs[wh][:, dc:dc + 1])
            nc.vector.tensor_tensor(out=t1[:], in0=t1[:], in1=t2[:], op=ALU.add)
            gt = gsm.tile([128, 224], F32, tag=f"gate_{gate}{dc}")
            nc.scalar.activation(gt[:], t1[:], AF.Sigmoid,
                                 bias=vecs[bi][:, dc:dc + 1])
            per_dc.append(gt)
        gates[gate] = per_dc

    # ================= phase 3: elementwise + store =================
    for j in range(BL):
        xsp = load_transposed(j, d_sp, xt_sp, "xsp", F32, False)
        stT, hidT = [], []
        for dc in range(NDC):
            g_sb = ew.tile([128, PIX], F32, tag="g_ret")
            nc.sync.dma_start(g_sb[:], g_parked[(j, dc)][:])
            st = outb.tile([128, PIX], F32, tag="stT")
            hd = outb.tile([128, PIX], F32, tag="hidT")
            stT.append(st)
            hidT.append(hd)
            for wi in range(NW):
                h0 = 1 + wi * WROWS
                base = (h0 - 1) * W
                t0 = j * H + (h0 - 1)

                def gw(gate):
                    return gates[gate][dc][:, t0:t0 + WROWS].to_broadcast(
                        [128, WROWS, W])

                sp3 = _r3(xsp[dc][:, base:base + WN], W)
                g3 = _r3(g_sb[:, base:base + WN], W)
                st3 = _r3(st[:, base:base + WN], W)
                hd3 = _r3(hd[:, base:base + WN], W)
                s1 = ew.tile([128, WN], F32, tag="s1")
                s13 = _r3(s1[:], W)
                nc.gpsimd.tensor_tensor(out=s13, in0=sp3, in1=gw("f"),
                                        op=ALU.mult)
                s2 = ew.tile([128, WN], F32, tag="s2")
                s23 = _r3(s2[:], W)
                nc.gpsimd.tensor_tensor(out=s23, in0=g3, in1=gw("i"),
                                        op=ALU.mult)
                nc.gpsimd.tensor_tensor(out=st3, in0=s13, in1=s23, op=ALU.add)
                th = ew.tile([128, WN], F32, tag="th")
                nc.scalar.activation(th[:], st[:, base:base + WN], AF.Tanh)
                nc.gpsimd.tensor_tensor(out=hd3, in0=_r3(th[:], W),
                                        in1=gw("o"), op=ALU.mult)

        for dname, buf in ((d_st, stT), (d_hid, hidT)):
            for g7 in range(NPG):
                nat = nato.tile([128, 256], F32, tag="natout")
                for dc in range(NDC):
                    pt = ps_tr.tile([PG, 128], F32, tag="ptr")
                    nc.tensor.transpose(pt[:], buf[dc][:, g7 * PG:(g7 + 1) * PG],
                                        ident[:])
                    nc.vector.tensor_copy(nat[0:PG, dc * 128:(dc + 1) * 128],
                                          pt[:])
                nc.sync.dma_start(dname[j, g7 * PG:(g7 + 1) * PG, :],
                                  nat[0:PG, :])

    ctx.close()


# ---------------------------------------------------------------------------
_NC_CACHE = None


def _get_nc():
    global _NC_CACHE
    if _NC_CACHE is None:
        nc = build_nc()
        _split_excess_sem_waits(nc)
        _NC_CACHE = nc
    return _NC_CACHE


def _gate_idx(core):
    idx = np.empty((4, 224, 1), np.int32)
    for j in range(BL):
        for hh in range(H):
            t = j * H + hh
            sel_b = (H * (BL * core + j) + hh) % B
            cp, bp = sel_b // BL, sel_b % BL
            for g in range(4):
                idx[g, t, 0] = cp * 32 + g * 8 + bp
    return idx


def _make_in_maps(inputs):
    f32 = np.float32
    ident = np.eye(128, dtype=f32)
    shared = {
        "wconvInput": np.ascontiguousarray(inputs["wconvInput"], dtype=f32),
        "wconvHidden": np.ascontiguousarray(inputs["wconvHidden"], dtype=f32),
        "gateBias": np.ascontiguousarray(inputs["gateBias"], dtype=f32).reshape(PIX, D),
        "identity": ident,
    }
    for nm in ("wxi", "whi", "inputBias", "wxf", "whf", "forgetBias",
               "wxo", "who", "outputBias"):
        shared[nm] = np.ascontiguousarray(inputs[nm], dtype=f32).reshape(D, 1)

    xin = np.ascontiguousarray(inputs["inputs"], dtype=f32).reshape(B, PIX, CIN)
    xsp = np.ascontiguousarray(inputs["state_prev"], dtype=f32).reshape(B, PIX, D)
    xhp = np.ascontiguousarray(inputs["hidden_prev"], dtype=f32).reshape(B, PIX, D)

    in_maps = []
    for k in range(N_CORES):
        sl = slice(k * BL, (k + 1) * BL)
        m = dict(shared)
        m["inputs"] = xin[sl]
        m["state_prev"] = xsp[sl]
        m["hidden_prev"] = xhp[sl]
        m["gate_idx"] = _gate_idx(k)
        in_maps.append(m)
    return in_maps


def kernel(**inputs):
    nc = _get_nc()
    in_maps = _make_in_maps(inputs)
    res = run_bass_kernel_spmd(nc, in_maps, core_ids=list(range(N_CORES)))
    hidden = np.concatenate([res.results[k]["hidden"] for k in range(N_CORES)],
                            axis=0).reshape(B, H, W, D)
    state = np.concatenate([res.results[k]["state"] for k in range(N_CORES)],
                           axis=0).reshape(B, H, W, D)
    return hidden, state
